# revision 11
# baseline (speedup 1.0000x reference)
"""Trainium2 Bass kernel for nn_Cross_Attention_Global (channel attention).

Math: the reference computes, per batch b and head h,
    q = emb @ Wq.T ; k = emb @ Wk.T ; v = emb @ Wv.T          (per-token projections)
    attn[b,h] = sum_n q[b,n,h,:] (x) k[b,n,h,:]               ([C,C] Gram-style reduction over N)
    attn      = softmax_rows(instancenorm(attn))
    z[b,n,c,h] = sum_d attn[b,h,c,d] v[b,n,h,d]
    out = z.reshape(.., C*H) @ Wo.T

Because the N-reduction is bilinear in emb, attn[b,h] = Wq_h G_b Wk_h^T with
G_b = emb_b^T emb_b (64x64).  And the output collapses to
    out_b = emb_b @ W_eff_b^T,
    W_eff_b^T = sum_h M_bh^T-contraction with Wo slices,  M_bh = attn_bh @ Wv_h.
So each core only needs the 64x64 Gram matrix, a handful of 64x64 matmuls +
instancenorm/softmax on [8,64,64], and one final [*,64]x[64,64] projection.

Sharding (8 cores): core c -> batch b=c//2, sequence half c%2. Each core
computes G over the FULL batch (the emb input is rolled so the core's own half
comes first), then projects only its first 8192 rows. No collectives needed.
"""

import numpy as np

import concourse.bass as bass
import concourse.tile as tile
from concourse import bacc
from concourse import mybir
from concourse import bass_isa
from concourse.bass_utils import run_bass_kernel_spmd
from concourse.masks import make_identity

B, N, C, H = 4, 16384, 64, 8
NHALF = N // 2          # 8192 tokens owned per core
EPS = 1e-5
FP = mybir.dt.float32

# emb tiling: GRP groups x TPG tiles x 128 tokens
TPG = 16                # 128-token tiles per group
GRP = N // (128 * TPG)  # 8 groups for G phase
OGRP = NHALF // (128 * TPG)  # 4 groups for the output phase


def build_kernel():
    nc = bacc.Bacc("TRN2", target_bir_lowering=False)
    emb_d = nc.dram_tensor("emb", [N, C], FP, kind="ExternalInput")
    wq_d = nc.dram_tensor("wq", [H * C, C], FP, kind="ExternalInput")
    wk_d = nc.dram_tensor("wk", [H * C, C], FP, kind="ExternalInput")
    wv_d = nc.dram_tensor("wv", [H * C, C], FP, kind="ExternalInput")
    wo_d = nc.dram_tensor("wo", [C, H * C], FP, kind="ExternalInput")
    out_d = nc.dram_tensor("out", [NHALF, C], FP, kind="ExternalOutput")

    emb_t = emb_d[:, :].rearrange("(g t p) c -> g p t c", t=TPG, p=128)
    out_t = out_d[:, :].rearrange("(g t p) c -> g p t c", t=TPG, p=128)

    with tile.TileContext(nc) as tc:
        with (
            tc.tile_pool(name="consts", bufs=1) as consts,
            tc.tile_pool(name="emb", bufs=1) as embp,
            tc.tile_pool(name="small", bufs=1) as small,
            tc.tile_pool(name="ostage", bufs=2) as ostagep,
            tc.tile_pool(name="scratch", bufs=4) as scratch,
        ):
            # ---- constants / weights ----
            identity = consts.tile([128, 128], FP)
            make_identity(nc, identity)

            wq_sb = consts.tile([128, 4, C], FP, tag="wq")
            wk_sb = consts.tile([128, 4, C], FP, tag="wk")
            wv_sb = consts.tile([C, H, C], FP, tag="wv")
            wo_sb = consts.tile([C, H * C], FP, tag="wo")
            nc.sync.dma_start(out=wq_sb, in_=wq_d[:, :].rearrange("(s p) c -> p s c", p=128))
            nc.sync.dma_start(out=wk_sb, in_=wk_d[:, :].rearrange("(s p) c -> p s c", p=128))
            nc.sync.dma_start(out=wv_sb, in_=wv_d[:, :].rearrange("(h d) e -> d h e", d=C))
            nc.sync.dma_start(out=wo_sb, in_=wo_d[:, :])

            # ---- load emb (resident in SBUF, reused by both phases) ----
            emb_sb = []
            for g in range(GRP):
                t_ = embp.tile([128, TPG, C], FP, tag=f"emb{g}")
                nc.sync.dma_start(out=t_, in_=emb_t[g])
                emb_sb.append(t_)

            with tc.tile_pool(name="psum1", bufs=1, space="PSUM") as psum1:
                # ---- weight transposes: WqT/WkT [cin=64, H*C=512] ----
                wqT = consts.tile([C, H * C], FP, tag="wqT")
                wkT = consts.tile([C, H * C], FP, tag="wkT")
                for (w_sb, wT) in ((wq_sb, wqT), (wk_sb, wkT)):
                    for s in range(4):
                        tp = psum1.tile([C, 128], FP, tag="misc")
                        nc.tensor.transpose(tp, w_sb[:, s, :], identity)
                        nc.scalar.copy(wT[:, bass.ts(s, 128)], tp)

                # WoT_byh[h][c,f] = Wo[f, c*H+h] transposed slices
                wo_str = wo_sb.rearrange("p (c h) -> p c h", h=H)
                woT = consts.tile([C, H, C], FP, tag="woT")
                for h in range(H):
                    tp = psum1.tile([C, C], FP, tag="misc")
                    nc.tensor.transpose(tp, wo_str[:, :, h], identity[:C, :C])
                    nc.scalar.copy(woT[:, h, :], tp)

                # ---- G = emb^T emb over all N (accumulate in PSUM) ----
                g_psum = psum1.tile([C, C], FP, tag="gpsum")
                nmm = GRP * TPG
                i = 0
                for g in range(GRP):
                    for t in range(TPG):
                        nc.tensor.matmul(
                            g_psum,
                            emb_sb[g][:, t, :],
                            emb_sb[g][:, t, :],
                            start=(i == 0),
                            stop=(i == nmm - 1),
                        )
                        i += 1
                g_sb = small.tile([C, C], FP, tag="g")
                nc.scalar.copy(g_sb, g_psum)

                # ---- attn (pre-norm) for all heads: [64(c), 8(h), 64(d)] ----
                # qt[l, h*64+c] = (Wq_h @ G)[c, l]
                qt_psum = psum1.tile([C, H * C], FP, tag="qt")
                nc.tensor.matmul(qt_psum, g_sb, wqT)
                qt_sb = small.tile([C, H * C], FP, tag="qts")
                nc.scalar.copy(qt_sb, qt_psum)

                attn_psum = psum1.tile([C, H, C], FP, tag="attn")
                for h in range(H):
                    nc.tensor.matmul(
                        attn_psum[:, h, :],
                        qt_sb[:, bass.ts(h, C)],
                        wkT[:, bass.ts(h, C)],
                    )

                # ---- instancenorm stats + softmax ----
                # per-(b,h) var over the whole 64x64 map; mean cancels in softmax
                st = small.tile([C, 2 * H], FP, tag="st")  # [sums | sumsqs]
                nc.vector.tensor_reduce(
                    st[:, 0:H], attn_psum, axis=mybir.AxisListType.X,
                    op=mybir.AluOpType.add,
                )
                sq = small.tile([C, H, C], FP, tag="sq")
                nc.scalar.square(sq, attn_psum)
                nc.vector.tensor_reduce(
                    st[:, H:2 * H], sq, axis=mybir.AxisListType.X,
                    op=mybir.AluOpType.add,
                )
                # negated row-max (per (c,h)) for stable softmax
                negmax = small.tile([C, H], FP, tag="negmax")
                nc.vector.tensor_reduce(
                    negmax, attn_psum, axis=mybir.AxisListType.X,
                    op=mybir.AluOpType.max, negate=True,
                )
                # cross-partition totals via PE (ones^T @ st), then compute
                # rstd on one partition and broadcast back via PE
                ones_col = consts.tile([C, 1], FP, tag="ones_col")
                nc.vector.memset(ones_col, 1.0)
                ones_row = consts.tile([1, C], FP, tag="ones_row")
                nc.vector.memset(ones_row, 1.0)
                tot_psum = psum1.tile([1, 2 * H], FP, tag="misc")
                nc.tensor.matmul(tot_psum, ones_col, st)
                mean_msq = small.tile([1, 2 * H], FP, tag="meanmsq")
                nc.vector.tensor_scalar_mul(mean_msq, tot_psum, 1.0 / (C * C))
                var = small.tile([1, H], FP, tag="var")
                nc.vector.tensor_mul(var, mean_msq[:, 0:H], mean_msq[:, 0:H])
                nc.vector.tensor_sub(var, mean_msq[:, H:2 * H], var)
                sd = small.tile([1, H], FP, tag="sd")
                eps_sb = small.tile([1, 1], FP, tag="eps")
                nc.vector.memset(eps_sb, EPS)
                nc.scalar.activation(sd, var, mybir.ActivationFunctionType.Sqrt, bias=eps_sb)
                rstd_row = small.tile([1, H], FP, tag="rstd_row")
                nc.vector.reciprocal(rstd_row, sd)
                bc_psum = psum1.tile([C, H], FP, tag="misc")
                nc.tensor.matmul(bc_psum, ones_row, rstd_row)
                rstd = small.tile([C, H], FP, tag="rstd")
                nc.vector.tensor_copy(rstd, bc_psum)
                # softmax: e = exp((x - max) * rstd); bias = negmax*rstd
                ebias = small.tile([C, H], FP, tag="ebias")
                nc.vector.tensor_mul(ebias, negmax, rstd)
                e_sb = small.tile([C, H, C], FP, tag="esb")
                sumexp = small.tile([C, H], FP, tag="sumexp")
                for h in range(H):
                    nc.scalar.activation(
                        e_sb[:, h, :], attn_psum[:, h, :],
                        mybir.ActivationFunctionType.Exp,
                        bias=ebias[:, h:h + 1], scale=rstd[:, h:h + 1],
                        accum_out=sumexp[:, h:h + 1],
                    )
                rsum = small.tile([C, H], FP, tag="rsum")
                nc.vector.reciprocal(rsum, sumexp)
                a_sb = small.tile([C, H, C], FP, tag="asb")
                for h in range(H):
                    nc.vector.tensor_scalar_mul(
                        a_sb[:, h, :], e_sb[:, h, :], rsum[:, h:h + 1]
                    )

                # ---- transpose attn, M = attn @ Wv_h, W_effT accumulation ----
                at_psum = psum1.tile([C, H, C], FP, tag="atp")
                for h in range(H):
                    nc.tensor.transpose(at_psum[:, h, :], a_sb[:, h, :], identity[:C, :C])
                at_sb = small.tile([C, H, C], FP, tag="at")
                nc.vector.tensor_copy(at_sb, at_psum)

                m_psum = psum1.tile([C, H, C], FP, tag="mp")
                for h in range(H):
                    nc.tensor.matmul(
                        m_psum[:, h, :],
                        at_sb[:, h, :],
                        wv_sb[:, h, :],
                    )
                m_sb = small.tile([C, H, C], FP, tag="m")
                nc.vector.tensor_copy(m_sb, m_psum)

                weff_psum = psum1.tile([C, C], FP, tag="misc")
                for h in range(H):
                    nc.tensor.matmul(
                        weff_psum, m_sb[:, h, :], woT[:, h, :],
                        start=(h == 0), stop=(h == H - 1),
                    )
                weffT = small.tile([C, C], FP, tag="weffT")
                nc.scalar.copy(weffT, weff_psum)

            # ---- phase 2: out = emb[:8192] @ W_eff^T ----
            with tc.tile_pool(name="psum2", bufs=3, space="PSUM") as psum2:
                for g in range(OGRP):
                    ostage = ostagep.tile([128, TPG, C], FP, tag="ostage")
                    for t in range(TPG):
                        tp = psum2.tile([C, 128], FP, tag="tp")
                        nc.tensor.transpose(tp, emb_sb[g][:, t, :], identity)
                        embT = scratch.tile([C, 128], FP, tag="embT")
                        nc.vector.tensor_copy(embT, tp)
                        op = psum2.tile([128, C], FP, tag="op")
                        nc.tensor.matmul(op, embT, weffT)
                        nc.vector.tensor_copy(ostage[:, t, :], op)
                    nc.sync.dma_start(out=out_t[g], in_=ostage)

    nc.finalize()
    return nc


_NC_CACHE = {}


def kernel(emb, Wq, Wk, Wv, Wo):
    if "nc" not in _NC_CACHE:
        _NC_CACHE["nc"] = build_kernel()
    nc = _NC_CACHE["nc"]

    emb = np.ascontiguousarray(emb, dtype=np.float32)
    in_maps = []
    for c in range(8):
        b, half = c // 2, c % 2
        e = emb[b]
        if half == 1:
            e = np.concatenate([e[NHALF:], e[:NHALF]], axis=0)
        in_maps.append({
            "emb": np.ascontiguousarray(e),
            "wq": np.ascontiguousarray(Wq, dtype=np.float32),
            "wk": np.ascontiguousarray(Wk, dtype=np.float32),
            "wv": np.ascontiguousarray(Wv, dtype=np.float32),
            "wo": np.ascontiguousarray(Wo, dtype=np.float32),
        })

    res = run_bass_kernel_spmd(nc, in_maps, core_ids=list(range(8))).results
    out = np.empty((B, N, C), np.float32)
    for c in range(8):
        b, half = c // 2, c % 2
        out[b, half * NHALF:(half + 1) * NHALF] = res[c]["out"]
    return out
